# revision 29
# baseline (speedup 1.0000x reference)
"""Bipolar self-attention on 8 Trainium2 NeuronCores.

Sharding: data-parallel over batch (B=2 -> 2 groups of 4 cores), tensor-
parallel over heads within a group (16 heads -> 4 heads/core). Each core:
  - projects its head-slice of Q/K transposed ([c, n] layout) and V natural,
    with the bipolar transform (q-0.5)*2 and the 1/sqrt(Dh) score scale
    folded into the projection weights/biases host-side,
  - computes S^T = Kb Qb^T per head tile-by-tile, exponentiates (softmax
    without max subtraction -- scores are O(10), exp is safe in fp32),
  - multiplies P^T by a [V_A | ones | V_B] stationary block: the PV matmul
    for head A uses cols 0-127 ([V_A | ones]) so PSUM rows 0-63 hold the
    attention output and rows 64-127 hold the softmax denominator already
    replicated across 64 partitions; head B uses cols 64-191 ([ones | V_B])
    with the roles of the row halves flipped.  Matmul cost depends only on
    the moving dim, so the denominator broadcast is free,
  - normalizes with one reciprocal_approx_fast + one tensor_mul straight
    from PSUM (no DRAM broadcast roundtrip, no PSUM evacuation copy),
  - applies its slice of the output projection (row-parallel).
Host sums the 4 partial outputs per batch and adds the bias terms.

All matmuls run in float32r (1 cycle/row at moving>=256).  The PE executes
in order, so independent projection / output-projection matmuls are
interleaved INTO the attention k-tile loops to fill the PE's exp-wait gaps,
and emission is pair-major (all 4 q-windows of head-pair 0, then of pair 1)
so the second pair's Q/K projection spreads across pair 0's ACT-bound slack.
"""

import ml_dtypes
import numpy as np

import concourse.bass as bass
import concourse.tile as tile
from concourse import bacc, mybir
from concourse.bass_utils import run_bass_kernel_spmd

D_MODEL = 1024
NHEAD = 16
HEAD_DIM = 64
B = 2
N = 2048
N_CORES = 8
HEADS_PER_CORE = NHEAD // (N_CORES // B)  # 4
C_LOC = HEADS_PER_CORE * HEAD_DIM  # 256

F32 = mybir.dt.float32
F32R = mybir.dt.float32r
BF16 = mybir.dt.bfloat16

_CACHE = {}


def build_nc():
    nc = bacc.Bacc("TRN2", target_bir_lowering=False, debug=False)

    xT = nc.dram_tensor("xT", [D_MODEL, N], F32R, kind="ExternalInput")
    wqT = nc.dram_tensor("wqT", [D_MODEL, C_LOC], F32R, kind="ExternalInput")
    wkT = nc.dram_tensor("wkT", [D_MODEL, C_LOC], F32R, kind="ExternalInput")
    wvT = nc.dram_tensor("wvT", [D_MODEL, C_LOC], F32R, kind="ExternalInput")
    woT = nc.dram_tensor("woT", [C_LOC, D_MODEL], BF16, kind="ExternalInput")
    bq = nc.dram_tensor("bq", [C_LOC], F32, kind="ExternalInput")
    bk = nc.dram_tensor("bk", [C_LOC], F32, kind="ExternalInput")
    y = nc.dram_tensor("y", [N, D_MODEL], F32, kind="ExternalOutput")

    NT = N // 128          # 16 k tiles
    DC = D_MODEL // 128    # 8 contraction chunks
    CT = C_LOC // 128      # 2 local-channel tiles (= head pairs)
    QW = 512               # q window width
    NW = N // QW           # 4 q windows

    with tile.TileContext(nc) as tc:
        with (
            tc.tile_pool(name="singles", bufs=1) as singles,
            tc.tile_pool(name="pt", bufs=4) as ptp,
            tc.tile_pool(name="rec", bufs=4) as recp,
            tc.tile_pool(name="yout", bufs=3) as youtp,
        ):
            # small biases first, then the weights/x slices the first
            # projection chain needs, so the PE can start ~6us in.
            bq_sb = singles.tile([128, CT], F32)
            nc.sync.dma_start(bq_sb[:], bq.ap().rearrange("(c p) -> p c", p=128))
            bk_sb = singles.tile([128, CT], F32)
            nc.sync.dma_start(bk_sb[:], bk.ap().rearrange("(c p) -> p c", p=128))
            wqT_sb = singles.tile([128, DC, C_LOC], F32R)
            nc.sync.dma_start(wqT_sb[:], wqT.ap().rearrange("(c p) m -> p c m", p=128))
            xT_sb = singles.tile([128, DC, N], F32R)
            xT_r = xT.ap().rearrange("(c p) n -> p c n", p=128)
            for dc in range(DC):
                nc.sync.dma_start(xT_sb[:, dc, 0:QW], xT_r[:, dc, 0:QW])
            wkT_sb = singles.tile([128, DC, C_LOC], F32R)
            nc.sync.dma_start(wkT_sb[:], wkT.ap().rearrange("(c p) m -> p c m", p=128))
            wvT_sb = singles.tile([128, DC, C_LOC], F32R)
            nc.sync.dma_start(wvT_sb[:], wvT.ap().rearrange("(c p) m -> p c m", p=128))
            for blk in range(1, NW):
                for dc in range(DC):
                    nc.sync.dma_start(
                        xT_sb[:, dc, blk * QW:(blk + 1) * QW],
                        xT_r[:, dc, blk * QW:(blk + 1) * QW],
                    )
            woT_sb = singles.tile([128, CT, D_MODEL], BF16)
            nc.sync.dma_start(woT_sb[:], woT.ap().rearrange("(c p) m -> p c m", p=128))

            qT_sb = singles.tile([128, CT, N], F32R)
            kT_sb = singles.tile([128, CT, N], F32R)
            # V stationary blocks: per (k-tile, pair, half) a [128, 128]
            # block [V_head (64) | ones (64)]: PV output rows 0:64 are the
            # attention output, rows 64:128 the softmax denominator
            # replicated across partitions (broadcast for free).
            v1_sb = singles.tile([128, NT, CT, 2, 128], BF16)
            ones_sb = singles.tile([128, 128], F32)
            nc.vector.memset(ones_sb[:], 1.0)
            for nt in range(NT):
                for pair in range(CT):
                    nc.vector.tensor_copy(
                        v1_sb[:, nt, pair, :, 64:128],
                        ones_sb[:].rearrange("p (h d) -> p h d", h=2),
                    )
            outT_sb = singles.tile([128, CT, N], BF16)

            # ---- emission helpers.  All PE work is emitted via closures so
            # the interleaving below is explicit.
            with (
                tc.tile_pool(name="ps512", bufs=2, space="PSUM") as psp,
                tc.tile_pool(name="st_ps", bufs=2, space="PSUM") as stp,
                tc.tile_pool(name="ov_ps", bufs=2, space="PSUM") as ovp,
            ):
                def qk_proj_part(w_sb, b_sb, dst, ct, nch, box, part):
                    # half of a 512-wide chunk (4 of 8 dc matmuls); the two
                    # halves share one PSUM accumulator via `box`.
                    if part == 0:
                        box["ps"] = psp.tile([128, 512], F32, tag="ps", name="projps")
                    ps = box["ps"]
                    for dc in range(part * 4, part * 4 + 4):
                        nc.tensor.matmul(
                            ps[:],
                            w_sb[:, dc, ct * 128:(ct + 1) * 128],
                            xT_sb[:, dc, nch * 512:(nch + 1) * 512],
                            start=(dc == 0),
                            stop=(dc == DC - 1),
                        )
                    if part == 1:
                        nc.vector.tensor_tensor(
                            dst[:, ct, nch * 512:(nch + 1) * 512],
                            ps[:],
                            b_sb[:, ct:ct + 1].to_broadcast((128, 512)),
                            mybir.AluOpType.add,
                        )

                def qk_proj_chunk(w_sb, b_sb, dst, ct, nch):
                    box = {}
                    qk_proj_part(w_sb, b_sb, dst, ct, nch, box, 0)
                    qk_proj_part(w_sb, b_sb, dst, ct, nch, box, 1)

                def v_proj_tile(nt):
                    # V natural: v[n, c] = sum_d xT[d, n] wvT[d, c], one n tile
                    ps = psp.tile([128, 512], F32, tag="ps")
                    for dc in range(DC):
                        nc.tensor.matmul(
                            ps[:, :C_LOC],
                            xT_sb[:, dc, nt * 128:(nt + 1) * 128],
                            wvT_sb[:, dc, :],
                            start=(dc == 0),
                            stop=(dc == DC - 1),
                        )
                    # scatter the 4 heads into their [pair, half] slots
                    nc.vector.tensor_copy(
                        v1_sb[:, nt].rearrange("p c h w -> p (c h) w")[:, :, 0:64],
                        ps[:, :C_LOC].rearrange("p (g d) -> p g d", g=4),
                    )

                def y_proj_part(nt, box, ct):
                    # one contraction step of the output projection for a
                    # 128-row tile; ct-outer so each stationary load feeds
                    # both 512-col chunks.
                    if ct == 0:
                        box["pss"] = [
                            psp.tile([128, 512], F32, tag="ps", name=f"yps{i}")
                            for i in range(D_MODEL // 512)
                        ]
                    pss = box["pss"]
                    for cok, ps in enumerate(pss):
                        nc.tensor.matmul(
                            ps[:],
                            outT_sb[:, ct, nt * 128:(nt + 1) * 128],
                            woT_sb[:, ct, cok * 512:(cok + 1) * 512],
                            start=(ct == 0),
                            stop=(ct == CT - 1),
                        )
                    if ct == CT - 1:
                        for cok, ps in enumerate(pss):
                            ys = youtp.tile([128, 512], F32, tag="ys")
                            nc.vector.tensor_copy(ys[:], ps[:])
                            nc.sync.dma_start(
                                y.ap()[nt * 128:(nt + 1) * 128,
                                       cok * 512:(cok + 1) * 512],
                                ys[:],
                            )

                def y_proj_tile(nt):
                    box = {}
                    for ct in range(CT):
                        y_proj_part(nt, box, ct)

                def attention_window(qq, pair, filler=None):
                    """One q-window of one head pair.  `filler()` is called
                    once per k-tile to emit interleaved PE work."""
                    q0 = qq * QW
                    ovA = ovp.tile([128, QW], F32, tag="ov")
                    ovB = ovp.tile([128, QW], F32, tag="ov")
                    for kt in range(NT):
                        st = stp.tile([128, 2 * QW], F32)
                        for half, p0 in ((0, 0), (1, 64)):
                            nc.tensor.matmul(
                                st[:, half * QW:(half + 1) * QW],
                                kT_sb[p0:p0 + 64, pair,
                                      kt * 128:(kt + 1) * 128],
                                qT_sb[p0:p0 + 64, pair, q0:q0 + QW],
                                start=True,
                                stop=True,
                            )
                        pt = ptp.tile([128, 2 * QW], BF16)
                        nc.scalar.activation(
                            pt[:], st[:], mybir.ActivationFunctionType.Exp
                        )
                        if filler is not None:
                            filler(kt)
                        # [V | ones] -> out rows 0:64, den rows 64:128
                        for half, ov in ((0, ovA), (1, ovB)):
                            nc.tensor.matmul(
                                ov[:],
                                v1_sb[:, kt, pair, half],
                                pt[:, half * QW:(half + 1) * QW],
                                start=(kt == 0),
                                stop=(kt == NT - 1),
                            )
                    # normalize: rec = 1/den (broadcast across partitions is
                    # already materialized), outT = out * rec.  The custom-DVE
                    # reciprocal misbehaves on PSUM input at partition offset
                    # 64, so bounce den through SBUF first (tensor_copy with
                    # cross partition offsets is fine).
                    for half, ov in ((0, ovA), (1, ovB)):
                        p0 = 64 * half
                        den = recp.tile([64, QW], F32, tag="den")
                        nc.vector.tensor_copy(den[:], ov[64:128, :])
                        rec = recp.tile([64, QW], F32, tag="rec")
                        nc.vector.reciprocal_approx_fast(rec[:], den[:])
                        nc.vector.tensor_mul(
                            outT_sb[p0:p0 + 64, pair, q0:q0 + QW],
                            ov[0:64, :],
                            rec[:],
                        )

                # ---- schedule -------------------------------------------
                # Warm-up: dummy fp32 matmuls on a zeroed scratch while the
                # first x-block DMAs land.  The PE's DVFS ramps from 0.65 to
                # 2.4 GHz only after ~3us of continuous work, so without
                # these the whole first projection runs at the low p-state.
                warm = singles.tile([128, 512], F32)
                nc.vector.memset(warm[:], 0.0)
                for _ in range(12):
                    wps = psp.tile([128, 512], F32, tag="ps", name="warm_ps")
                    nc.tensor.matmul(
                        wps[:, 0:256], warm[:, 0:128], warm[:, 0:256],
                        start=True, stop=True,
                    )

                # Window (0,0) starts right after the q/k projections of x
                # block 0; its filler emits the later k-projection chunks
                # (paced to the x block DMAs), the V tiles, and at the end
                # the q chunks for windows 1-3.
                qk_proj_chunk(wqT_sb, bq_sb, qT_sb, 0, 0)
                qk_proj_chunk(wkT_sb, bk_sb, kT_sb, 0, 0)
                v_proj_tile(0)
                v_proj_tile(1)

                def fill_w00(kt):
                    if kt % 4 == 2 and kt // 4 + 1 < NW:
                        qk_proj_chunk(wkT_sb, bk_sb, kT_sb, 0, kt // 4 + 1)
                    if kt + 2 < NT:
                        v_proj_tile(kt + 2)
                    if kt >= 13:
                        qk_proj_chunk(wqT_sb, bq_sb, qT_sb, 0, kt - 12)

                attention_window(0, 0, filler=fill_w00)

                # pair-0 windows 1..3; spread pair-1 Q/K projection across
                # their slack at half-chunk (4-matmul) granularity so single
                # filler units never displace a whole ACT period.
                proj1 = []
                for nch in range(NW):
                    for w, b, dst in ((wqT_sb, bq_sb, qT_sb),
                                      (wkT_sb, bk_sb, kT_sb)):
                        box = {}
                        proj1.append((w, b, dst, nch, box, 0))
                        proj1.append((w, b, dst, nch, box, 1))
                it_proj1 = iter(proj1)

                def fill_proj(kt):
                    if kt % 2 == 0:
                        args = next(it_proj1, None)
                        if args is not None:
                            w, b, dst, nch, box, part = args
                            qk_proj_part(w, b, dst, 1, nch, box, part)

                attention_window(1, 0, filler=fill_proj)
                attention_window(2, 0, filler=fill_proj)
                attention_window(3, 0, filler=fill_proj)

                # pair-1 windows with trailing y projection interleaved at
                # half-tile granularity (one contraction step per slot).
                # y window qq is ready once pair-1 window qq is normalized.
                def make_fill_y(qq_ready):
                    units = []
                    for nt in range(qq_ready * (QW // 128),
                                    (qq_ready + 1) * (QW // 128)):
                        box = {}
                        for ct in range(CT):
                            units.append((nt, box, ct))
                    it = iter(units)

                    def fill(kt):
                        if kt % 2 == 0:
                            args = next(it, None)
                            if args is not None:
                                y_proj_part(*args)
                    return fill

                attention_window(0, 1)
                attention_window(1, 1, filler=make_fill_y(0))
                attention_window(2, 1, filler=make_fill_y(1))
                attention_window(3, 1, filler=make_fill_y(2))
                for nt in range(3 * (QW // 128), NW * (QW // 128)):
                    y_proj_tile(nt)

    nc.compile()
    return nc


def kernel(x, Wq, bq, Wk, bk, Wv, bv, Wo, bo):
    x = np.asarray(x, dtype=np.float32)
    Wq = np.asarray(Wq, dtype=np.float32)
    Wk = np.asarray(Wk, dtype=np.float32)
    Wv = np.asarray(Wv, dtype=np.float32)
    Wo = np.asarray(Wo, dtype=np.float32)
    bq = np.asarray(bq, dtype=np.float32)
    bk = np.asarray(bk, dtype=np.float32)
    bv = np.asarray(bv, dtype=np.float32)
    bo = np.asarray(bo, dtype=np.float32)

    if "nc" not in _CACHE:
        _CACHE["nc"] = build_nc()
    nc = _CACHE["nc"]

    s = 2.0 / np.sqrt(8.0)  # fold bipolar *2 and score scale (1/8 split per side)
    in_maps = []
    for core in range(N_CORES):
        b = core // (N_CORES // B)
        g = core % (N_CORES // B)
        ch = slice(g * C_LOC, (g + 1) * C_LOC)
        in_maps.append({
            "xT": np.ascontiguousarray(x[b].T),
            "wqT": np.ascontiguousarray((s * Wq[ch, :]).T),
            "wkT": np.ascontiguousarray((s * Wk[ch, :]).T),
            "wvT": np.ascontiguousarray(Wv[ch, :].T),
            "woT": np.ascontiguousarray(Wo[:, ch].T).astype(ml_dtypes.bfloat16),
            "bq": ((2.0 * bq[ch] - 1.0) / np.sqrt(8.0)).astype(np.float32),
            "bk": ((2.0 * bk[ch] - 1.0) / np.sqrt(8.0)).astype(np.float32),
        })

    _CACHE["in_maps"] = in_maps
    res = run_bass_kernel_spmd(nc, in_maps, core_ids=list(range(N_CORES)))

    g_per_b = N_CORES // B
    const = (Wo @ bv + bo).astype(np.float32)  # bv folded through out-proj
    out = np.empty((B, N, D_MODEL), dtype=np.float32)
    for b in range(B):
        acc = res.results[b * g_per_b]["y"].astype(np.float32).copy()
        for g in range(1, g_per_b):
            acc += res.results[b * g_per_b + g]["y"]
        out[b] = acc + const
    return out


# revision 30
# speedup vs baseline: 1.1669x; 1.1669x over previous
"""Bipolar self-attention on 8 Trainium2 NeuronCores.

Sharding: data-parallel over batch (B=2 -> 2 groups of 4 cores), tensor-
parallel over heads within a group (16 heads -> 4 heads/core). Each core:
  - projects its head-slice of Q/K transposed ([c, n] layout) and V natural,
    with the bipolar transform (q-0.5)*2 and the 1/sqrt(Dh) score scale
    folded into the projection weights/biases host-side,
  - computes S^T = Kb Qb^T per head tile-by-tile, exponentiates (softmax
    without max subtraction -- scores are O(10), exp is safe in fp32),
  - multiplies P^T by a [V_A | ones | V_B] stationary block: the PV matmul
    for head A uses cols 0-127 ([V_A | ones]) so PSUM rows 0-63 hold the
    attention output and rows 64-127 hold the softmax denominator already
    replicated across 64 partitions; head B uses cols 64-191 ([ones | V_B])
    with the roles of the row halves flipped.  Matmul cost depends only on
    the moving dim, so the denominator broadcast is free,
  - normalizes with one reciprocal_approx_fast + one tensor_mul straight
    from PSUM (no DRAM broadcast roundtrip, no PSUM evacuation copy),
  - applies its slice of the output projection (row-parallel).
Host sums the 4 partial outputs per batch and adds the bias terms.

All matmuls run in float32r (1 cycle/row at moving>=256).  The PE executes
in order, so independent projection / output-projection matmuls are
interleaved INTO the attention k-tile loops to fill the PE's exp-wait gaps,
and emission is pair-major (all 4 q-windows of head-pair 0, then of pair 1)
so the second pair's Q/K projection spreads across pair 0's ACT-bound slack.
"""

import ml_dtypes
import numpy as np

import concourse.bass as bass
import concourse.tile as tile
from concourse import bacc, mybir
from concourse.bass_utils import run_bass_kernel_spmd

D_MODEL = 1024
NHEAD = 16
HEAD_DIM = 64
B = 2
N = 2048
N_CORES = 8
HEADS_PER_CORE = NHEAD // (N_CORES // B)  # 4
C_LOC = HEADS_PER_CORE * HEAD_DIM  # 256

F32 = mybir.dt.float32
F32R = mybir.dt.float32r
BF16 = mybir.dt.bfloat16

_CACHE = {}


def build_nc():
    nc = bacc.Bacc("TRN2", target_bir_lowering=False, debug=False)

    xT = nc.dram_tensor("xT", [D_MODEL, N], F32R, kind="ExternalInput")
    wqT = nc.dram_tensor("wqT", [D_MODEL, C_LOC], F32R, kind="ExternalInput")
    wkT = nc.dram_tensor("wkT", [D_MODEL, C_LOC], F32R, kind="ExternalInput")
    wvT = nc.dram_tensor("wvT", [D_MODEL, C_LOC], F32R, kind="ExternalInput")
    woT = nc.dram_tensor("woT", [C_LOC, D_MODEL], BF16, kind="ExternalInput")
    bq = nc.dram_tensor("bq", [C_LOC], F32, kind="ExternalInput")
    bk = nc.dram_tensor("bk", [C_LOC], F32, kind="ExternalInput")
    y = nc.dram_tensor("y", [N, D_MODEL], F32, kind="ExternalOutput")

    NT = N // 128          # 16 k tiles
    DC = D_MODEL // 128    # 8 contraction chunks
    CT = C_LOC // 128      # 2 local-channel tiles (= head pairs)
    QW = 512               # q window width
    NW = N // QW           # 4 q windows

    with tile.TileContext(nc) as tc:
        with (
            tc.tile_pool(name="singles", bufs=1) as singles,
            tc.tile_pool(name="pt", bufs=4) as ptp,
            tc.tile_pool(name="rec", bufs=4) as recp,
            tc.tile_pool(name="yout", bufs=3) as youtp,
        ):
            # small biases first, then the weights/x slices the first
            # projection chain needs, so the PE can start ~6us in.
            bq_sb = singles.tile([128, CT], F32)
            nc.sync.dma_start(bq_sb[:], bq.ap().rearrange("(c p) -> p c", p=128))
            bk_sb = singles.tile([128, CT], F32)
            nc.sync.dma_start(bk_sb[:], bk.ap().rearrange("(c p) -> p c", p=128))
            wqT_sb = singles.tile([128, DC, C_LOC], F32R)
            nc.sync.dma_start(wqT_sb[:], wqT.ap().rearrange("(c p) m -> p c m", p=128))
            xT_sb = singles.tile([128, DC, N], F32R)
            xT_r = xT.ap().rearrange("(c p) n -> p c n", p=128)
            for dc in range(DC):
                nc.sync.dma_start(xT_sb[:, dc, 0:QW], xT_r[:, dc, 0:QW])
            wkT_sb = singles.tile([128, DC, C_LOC], F32R)
            nc.sync.dma_start(wkT_sb[:], wkT.ap().rearrange("(c p) m -> p c m", p=128))
            wvT_sb = singles.tile([128, DC, C_LOC], F32R)
            nc.sync.dma_start(wvT_sb[:], wvT.ap().rearrange("(c p) m -> p c m", p=128))
            for blk in range(1, NW):
                for dc in range(DC):
                    nc.sync.dma_start(
                        xT_sb[:, dc, blk * QW:(blk + 1) * QW],
                        xT_r[:, dc, blk * QW:(blk + 1) * QW],
                    )
            woT_sb = singles.tile([128, CT, D_MODEL], BF16)
            nc.sync.dma_start(woT_sb[:], woT.ap().rearrange("(c p) m -> p c m", p=128))

            qT_sb = singles.tile([128, CT, N], F32R)
            kT_sb = singles.tile([128, CT, N], F32R)
            # V stationary blocks: per (k-tile, pair, half) a [128, 128]
            # block [V_head (64) | ones (64)]: PV output rows 0:64 are the
            # attention output, rows 64:128 the softmax denominator
            # replicated across partitions (broadcast for free).
            v1_sb = singles.tile([128, NT, CT, 2, 128], BF16)
            ones_sb = singles.tile([128, 128], F32)
            nc.vector.memset(ones_sb[:], 1.0)
            for nt in range(NT):
                for pair in range(CT):
                    nc.vector.tensor_copy(
                        v1_sb[:, nt, pair, :, 64:128],
                        ones_sb[:].rearrange("p (h d) -> p h d", h=2),
                    )
            outT_sb = singles.tile([128, CT, N], BF16)

            # ---- emission helpers.  All PE work is emitted via closures so
            # the interleaving below is explicit.
            with (
                tc.tile_pool(name="ps512", bufs=2, space="PSUM") as psp,
                tc.tile_pool(name="st_ps", bufs=2, space="PSUM") as stp,
                tc.tile_pool(name="ov_ps", bufs=2, space="PSUM") as ovp,
            ):
                def qk_proj_chunk(w_sb, b_sb, dst, ct, nch):
                    # one 512-wide chunk: 8 matmuls + bias add
                    ps = psp.tile([128, 512], F32, tag="ps")
                    for dc in range(DC):
                        nc.tensor.matmul(
                            ps[:],
                            w_sb[:, dc, ct * 128:(ct + 1) * 128],
                            xT_sb[:, dc, nch * 512:(nch + 1) * 512],
                            start=(dc == 0),
                            stop=(dc == DC - 1),
                        )
                    nc.vector.tensor_tensor(
                        dst[:, ct, nch * 512:(nch + 1) * 512],
                        ps[:],
                        b_sb[:, ct:ct + 1].to_broadcast((128, 512)),
                        mybir.AluOpType.add,
                    )

                def v_proj_tile(nt):
                    # V natural: v[n, c] = sum_d xT[d, n] wvT[d, c], one n tile
                    ps = psp.tile([128, 512], F32, tag="ps")
                    for dc in range(DC):
                        nc.tensor.matmul(
                            ps[:, :C_LOC],
                            xT_sb[:, dc, nt * 128:(nt + 1) * 128],
                            wvT_sb[:, dc, :],
                            start=(dc == 0),
                            stop=(dc == DC - 1),
                        )
                    # scatter the 4 heads into their [pair, half] slots
                    nc.vector.tensor_copy(
                        v1_sb[:, nt].rearrange("p c h w -> p (c h) w")[:, :, 0:64],
                        ps[:, :C_LOC].rearrange("p (g d) -> p g d", g=4),
                    )

                def y_proj_tile(nt):
                    # output projection for one 128-row tile: ct-outer so each
                    # stationary load feeds both 512-col chunks.
                    pss = [psp.tile([128, 512], F32, tag="ps", name=f"yps{i}")
                           for i in range(D_MODEL // 512)]
                    for ct in range(CT):
                        for cok, ps in enumerate(pss):
                            nc.tensor.matmul(
                                ps[:],
                                outT_sb[:, ct, nt * 128:(nt + 1) * 128],
                                woT_sb[:, ct, cok * 512:(cok + 1) * 512],
                                start=(ct == 0),
                                stop=(ct == CT - 1),
                            )
                    for cok, ps in enumerate(pss):
                        ys = youtp.tile([128, 512], F32, tag="ys")
                        nc.vector.tensor_copy(ys[:], ps[:])
                        nc.sync.dma_start(
                            y.ap()[nt * 128:(nt + 1) * 128,
                                   cok * 512:(cok + 1) * 512],
                            ys[:],
                        )

                def attention_window(qq, pair, filler=None):
                    """One q-window of one head pair.  `filler()` is called
                    once per k-tile to emit interleaved PE work."""
                    q0 = qq * QW
                    ovA = ovp.tile([128, QW], F32, tag="ov")
                    ovB = ovp.tile([128, QW], F32, tag="ov")
                    for kt in range(NT):
                        st = stp.tile([128, 2 * QW], F32)
                        for half, p0 in ((0, 0), (1, 64)):
                            nc.tensor.matmul(
                                st[:, half * QW:(half + 1) * QW],
                                kT_sb[p0:p0 + 64, pair,
                                      kt * 128:(kt + 1) * 128],
                                qT_sb[p0:p0 + 64, pair, q0:q0 + QW],
                                start=True,
                                stop=True,
                            )
                        pt = ptp.tile([128, 2 * QW], BF16)
                        nc.scalar.activation(
                            pt[:], st[:], mybir.ActivationFunctionType.Exp
                        )
                        if filler is not None:
                            filler(kt)
                        # [V | ones] -> out rows 0:64, den rows 64:128
                        for half, ov in ((0, ovA), (1, ovB)):
                            nc.tensor.matmul(
                                ov[:],
                                v1_sb[:, kt, pair, half],
                                pt[:, half * QW:(half + 1) * QW],
                                start=(kt == 0),
                                stop=(kt == NT - 1),
                            )
                    # normalize: rec = 1/den (broadcast across partitions is
                    # already materialized), outT = out * rec.  The custom-DVE
                    # reciprocal misbehaves on PSUM input at partition offset
                    # 64, so bounce den through SBUF first (tensor_copy with
                    # cross partition offsets is fine).
                    for half, ov in ((0, ovA), (1, ovB)):
                        p0 = 64 * half
                        den = recp.tile([64, QW], F32, tag="den")
                        nc.vector.tensor_copy(den[:], ov[64:128, :])
                        rec = recp.tile([64, QW], F32, tag="rec")
                        nc.vector.reciprocal_approx_fast(rec[:], den[:])
                        nc.vector.tensor_mul(
                            outT_sb[p0:p0 + 64, pair, q0:q0 + QW],
                            ov[0:64, :],
                            rec[:],
                        )

                # ---- schedule -------------------------------------------
                # Window (0,0) starts right after the q/k projections of x
                # block 0; its filler emits the later k-projection chunks
                # (paced to the x block DMAs), the V tiles, and at the end
                # the q chunks for windows 1-3.
                qk_proj_chunk(wqT_sb, bq_sb, qT_sb, 0, 0)
                qk_proj_chunk(wkT_sb, bk_sb, kT_sb, 0, 0)
                v_proj_tile(0)
                v_proj_tile(1)

                def fill_w00(kt):
                    if kt % 4 == 2 and kt // 4 + 1 < NW:
                        qk_proj_chunk(wkT_sb, bk_sb, kT_sb, 0, kt // 4 + 1)
                    if kt + 2 < NT:
                        v_proj_tile(kt + 2)
                    if kt >= 13:
                        qk_proj_chunk(wqT_sb, bq_sb, qT_sb, 0, kt - 12)

                attention_window(0, 0, filler=fill_w00)

                # pair-0 windows 1..3; spread pair-1 Q/K projection (8
                # chunks) across their slack, with a unit at kt=0 so the PE
                # has work while the previous window's normalize frees ov.
                proj1 = []
                for nch in range(NW):
                    proj1.append((wqT_sb, bq_sb, qT_sb, nch))
                    proj1.append((wkT_sb, bk_sb, kT_sb, nch))

                def make_fill_proj(chunks):
                    it = iter(chunks)

                    def fill(kt):
                        if kt % 4 == 0:
                            args = next(it, None)
                            if args is not None:
                                w, b, dst, nch = args
                                qk_proj_chunk(w, b, dst, 1, nch)
                    return fill

                fp = make_fill_proj(proj1)
                attention_window(1, 0, filler=fp)
                attention_window(2, 0, filler=fp)
                attention_window(3, 0, filler=fp)

                # pair-1 windows with trailing y projection interleaved.
                # y window qq is ready once pair-1 window qq is normalized.
                def make_fill_y(qq_ready):
                    chunks = list(range(qq_ready * (QW // 128),
                                        (qq_ready + 1) * (QW // 128)))
                    it = iter(chunks)

                    def fill(kt):
                        if kt % 4 == 0:
                            args = next(it, None)
                            if args is not None:
                                y_proj_tile(args)
                    return fill

                attention_window(0, 1)
                attention_window(1, 1, filler=make_fill_y(0))
                attention_window(2, 1, filler=make_fill_y(1))
                attention_window(3, 1, filler=make_fill_y(2))
                for nt in range(3 * (QW // 128), NW * (QW // 128)):
                    y_proj_tile(nt)

    nc.compile()
    return nc


def kernel(x, Wq, bq, Wk, bk, Wv, bv, Wo, bo):
    x = np.asarray(x, dtype=np.float32)
    Wq = np.asarray(Wq, dtype=np.float32)
    Wk = np.asarray(Wk, dtype=np.float32)
    Wv = np.asarray(Wv, dtype=np.float32)
    Wo = np.asarray(Wo, dtype=np.float32)
    bq = np.asarray(bq, dtype=np.float32)
    bk = np.asarray(bk, dtype=np.float32)
    bv = np.asarray(bv, dtype=np.float32)
    bo = np.asarray(bo, dtype=np.float32)

    if "nc" not in _CACHE:
        _CACHE["nc"] = build_nc()
    nc = _CACHE["nc"]

    s = 2.0 / np.sqrt(8.0)  # fold bipolar *2 and score scale (1/8 split per side)
    in_maps = []
    for core in range(N_CORES):
        b = core // (N_CORES // B)
        g = core % (N_CORES // B)
        ch = slice(g * C_LOC, (g + 1) * C_LOC)
        in_maps.append({
            "xT": np.ascontiguousarray(x[b].T),
            "wqT": np.ascontiguousarray((s * Wq[ch, :]).T),
            "wkT": np.ascontiguousarray((s * Wk[ch, :]).T),
            "wvT": np.ascontiguousarray(Wv[ch, :].T),
            "woT": np.ascontiguousarray(Wo[:, ch].T).astype(ml_dtypes.bfloat16),
            "bq": ((2.0 * bq[ch] - 1.0) / np.sqrt(8.0)).astype(np.float32),
            "bk": ((2.0 * bk[ch] - 1.0) / np.sqrt(8.0)).astype(np.float32),
        })

    _CACHE["in_maps"] = in_maps
    res = run_bass_kernel_spmd(nc, in_maps, core_ids=list(range(N_CORES)))

    g_per_b = N_CORES // B
    const = (Wo @ bv + bo).astype(np.float32)  # bv folded through out-proj
    out = np.empty((B, N, D_MODEL), dtype=np.float32)
    for b in range(B):
        acc = res.results[b * g_per_b]["y"].astype(np.float32).copy()
        for g in range(1, g_per_b):
            acc += res.results[b * g_per_b + g]["y"]
        out[b] = acc + const
    return out


# revision 34
# speedup vs baseline: 1.1688x; 1.0016x over previous
"""Bipolar self-attention on 8 Trainium2 NeuronCores.

Sharding: data-parallel over batch (B=2 -> 2 groups of 4 cores), tensor-
parallel over heads within a group (16 heads -> 4 heads/core). Each core:
  - projects its head-slice of Q/K transposed ([c, n] layout) and V natural,
    with the bipolar transform (q-0.5)*2 and the 1/sqrt(Dh) score scale
    folded into the projection weights/biases host-side,
  - computes S^T = Kb Qb^T per head tile-by-tile, exponentiates (softmax
    without max subtraction -- scores are O(10), exp is safe in fp32),
  - multiplies P^T by a per-(pair, half) [V | ones] stationary block (128
    cols) so PSUM rows 0-63 hold the attention output and rows 64-127 hold
    the softmax denominator already replicated across 64 partitions.
    Matmul cost depends only on the moving dim, so this denominator
    broadcast is free,
  - normalizes with tensor_copy (den -> SBUF; the custom-DVE reciprocal
    misreads PSUM at partition offset 64) + reciprocal_approx_fast +
    tensor_mul (no DRAM broadcast roundtrip, no PSUM evacuation copy),
  - applies its slice of the output projection (row-parallel).
Host sums the 4 partial outputs per batch and adds the bias terms.

Score path (x, Wq/Wk, Q^T/K^T) stays float32r (1 cycle/row at moving>=256;
bf16 there costs ~1e-2 of the 2e-2 error budget for no cycle gain); the
post-softmax path (P, V, out, Wo) is bf16, which cuts SBUF/DMA energy and
measurably reduces DVFS throttling.  The PE executes in order, so
projection / output-projection matmuls are interleaved INTO the attention
k-tile loops (window (0,0) weaves the k-projection chunks paced to the x
block DMAs); emission is pair-major, with y(qq-1) tiles filling the pair-1
windows.  Denser schedules (v7/v9 experiments) stretched ACT exp execution
~20% via shared power/SBUF-port pressure and were net slower.
"""

import ml_dtypes
import numpy as np

import concourse.bass as bass
import concourse.tile as tile
from concourse import bacc, mybir
from concourse.bass_utils import run_bass_kernel_spmd

D_MODEL = 1024
NHEAD = 16
HEAD_DIM = 64
B = 2
N = 2048
N_CORES = 8
HEADS_PER_CORE = NHEAD // (N_CORES // B)  # 4
C_LOC = HEADS_PER_CORE * HEAD_DIM  # 256

F32 = mybir.dt.float32
F32R = mybir.dt.float32r
BF16 = mybir.dt.bfloat16

_CACHE = {}


def build_nc():
    nc = bacc.Bacc("TRN2", target_bir_lowering=False, debug=False)

    xT = nc.dram_tensor("xT", [D_MODEL, N], F32R, kind="ExternalInput")
    wqT = nc.dram_tensor("wqT", [D_MODEL, C_LOC], F32R, kind="ExternalInput")
    wkT = nc.dram_tensor("wkT", [D_MODEL, C_LOC], F32R, kind="ExternalInput")
    wvT = nc.dram_tensor("wvT", [D_MODEL, C_LOC], F32R, kind="ExternalInput")
    woT = nc.dram_tensor("woT", [C_LOC, D_MODEL], BF16, kind="ExternalInput")
    bq = nc.dram_tensor("bq", [C_LOC], F32, kind="ExternalInput")
    bk = nc.dram_tensor("bk", [C_LOC], F32, kind="ExternalInput")
    y = nc.dram_tensor("y", [N, D_MODEL], F32, kind="ExternalOutput")

    NT = N // 128          # 16 k tiles
    DC = D_MODEL // 128    # 8 contraction chunks
    CT = C_LOC // 128      # 2 local-channel tiles (= head pairs)
    QW = 512               # q window width
    NW = N // QW           # 4 q windows

    with tile.TileContext(nc) as tc:
        with (
            tc.tile_pool(name="singles", bufs=1) as singles,
            tc.tile_pool(name="pt", bufs=4) as ptp,
            tc.tile_pool(name="rec", bufs=4) as recp,
            tc.tile_pool(name="yout", bufs=3) as youtp,
        ):
            # small biases first, then the weights/x slices the first
            # projection chain needs, so the PE can start ~6us in.
            bq_sb = singles.tile([128, CT], F32)
            nc.sync.dma_start(bq_sb[:], bq.ap().rearrange("(c p) -> p c", p=128))
            bk_sb = singles.tile([128, CT], F32)
            nc.sync.dma_start(bk_sb[:], bk.ap().rearrange("(c p) -> p c", p=128))
            wqT_sb = singles.tile([128, DC, C_LOC], F32R)
            nc.sync.dma_start(wqT_sb[:], wqT.ap().rearrange("(c p) m -> p c m", p=128))
            xT_sb = singles.tile([128, DC, N], F32R)
            xT_r = xT.ap().rearrange("(c p) n -> p c n", p=128)
            for dc in range(DC):
                nc.sync.dma_start(xT_sb[:, dc, 0:QW], xT_r[:, dc, 0:QW])
            wkT_sb = singles.tile([128, DC, C_LOC], F32R)
            nc.sync.dma_start(wkT_sb[:], wkT.ap().rearrange("(c p) m -> p c m", p=128))
            wvT_sb = singles.tile([128, DC, C_LOC], F32R)
            nc.sync.dma_start(wvT_sb[:], wvT.ap().rearrange("(c p) m -> p c m", p=128))
            for blk in range(1, NW):
                for dc in range(DC):
                    nc.sync.dma_start(
                        xT_sb[:, dc, blk * QW:(blk + 1) * QW],
                        xT_r[:, dc, blk * QW:(blk + 1) * QW],
                    )
            woT_sb = singles.tile([128, CT, D_MODEL], BF16)
            nc.sync.dma_start(woT_sb[:], woT.ap().rearrange("(c p) m -> p c m", p=128))

            qT_sb = singles.tile([128, CT, N], F32R)
            kT_sb = singles.tile([128, CT, N], F32R)
            # V stationary blocks: per (k-tile, pair, half) a [128, 128]
            # block [V_head (64) | ones (64)]: PV output rows 0:64 are the
            # attention output, rows 64:128 the softmax denominator
            # replicated across partitions (broadcast for free).
            v1_sb = singles.tile([128, NT, CT, 2, 128], BF16)
            ones_sb = singles.tile([128, 128], F32)
            nc.vector.memset(ones_sb[:], 1.0)
            for nt in range(NT):
                for pair in range(CT):
                    nc.vector.tensor_copy(
                        v1_sb[:, nt, pair, :, 64:128],
                        ones_sb[:].rearrange("p (h d) -> p h d", h=2),
                    )
            outT_sb = singles.tile([128, CT, N], BF16)

            # ---- emission helpers.  All PE work is emitted via closures so
            # the interleaving below is explicit.
            with (
                tc.tile_pool(name="ps512", bufs=2, space="PSUM") as psp,
                tc.tile_pool(name="st_ps", bufs=2, space="PSUM") as stp,
                tc.tile_pool(name="ov_ps", bufs=2, space="PSUM") as ovp,
            ):
                def qk_proj_chunk(w_sb, b_sb, dst, ct, nch):
                    # one 512-wide chunk: 8 matmuls + bias add
                    ps = psp.tile([128, 512], F32, tag="ps")
                    for dc in range(DC):
                        nc.tensor.matmul(
                            ps[:],
                            w_sb[:, dc, ct * 128:(ct + 1) * 128],
                            xT_sb[:, dc, nch * 512:(nch + 1) * 512],
                            start=(dc == 0),
                            stop=(dc == DC - 1),
                        )
                    nc.vector.tensor_tensor(
                        dst[:, ct, nch * 512:(nch + 1) * 512],
                        ps[:],
                        b_sb[:, ct:ct + 1].to_broadcast((128, 512)),
                        mybir.AluOpType.add,
                    )

                def v_proj_tile(nt):
                    # V natural: v[n, c] = sum_d xT[d, n] wvT[d, c], one n tile
                    ps = psp.tile([128, 512], F32, tag="ps")
                    for dc in range(DC):
                        nc.tensor.matmul(
                            ps[:, :C_LOC],
                            xT_sb[:, dc, nt * 128:(nt + 1) * 128],
                            wvT_sb[:, dc, :],
                            start=(dc == 0),
                            stop=(dc == DC - 1),
                        )
                    # scatter the 4 heads into their [pair, half] slots
                    nc.vector.tensor_copy(
                        v1_sb[:, nt].rearrange("p c h w -> p (c h) w")[:, :, 0:64],
                        ps[:, :C_LOC].rearrange("p (g d) -> p g d", g=4),
                    )

                def y_proj_tile(nt):
                    # output projection for one 128-row tile: ct-outer so each
                    # stationary load feeds both 512-col chunks.
                    pss = [psp.tile([128, 512], F32, tag="ps", name=f"yps{i}")
                           for i in range(D_MODEL // 512)]
                    for ct in range(CT):
                        for cok, ps in enumerate(pss):
                            nc.tensor.matmul(
                                ps[:],
                                outT_sb[:, ct, nt * 128:(nt + 1) * 128],
                                woT_sb[:, ct, cok * 512:(cok + 1) * 512],
                                start=(ct == 0),
                                stop=(ct == CT - 1),
                            )
                    for cok, ps in enumerate(pss):
                        ys = youtp.tile([128, 512], F32, tag="ys")
                        nc.vector.tensor_copy(ys[:], ps[:])
                        nc.sync.dma_start(
                            y.ap()[nt * 128:(nt + 1) * 128,
                                   cok * 512:(cok + 1) * 512],
                            ys[:],
                        )

                def attention_window(qq, pair, filler=None):
                    """One q-window of one head pair.  `filler()` is called
                    once per k-tile to emit interleaved PE work."""
                    q0 = qq * QW
                    ovA = ovp.tile([128, QW], F32, tag="ov")
                    ovB = ovp.tile([128, QW], F32, tag="ov")
                    for kt in range(NT):
                        st = stp.tile([128, 2 * QW], F32)
                        for half, p0 in ((0, 0), (1, 64)):
                            nc.tensor.matmul(
                                st[:, half * QW:(half + 1) * QW],
                                kT_sb[p0:p0 + 64, pair,
                                      kt * 128:(kt + 1) * 128],
                                qT_sb[p0:p0 + 64, pair, q0:q0 + QW],
                                start=True,
                                stop=True,
                            )
                        pt = ptp.tile([128, 2 * QW], BF16)
                        nc.scalar.activation(
                            pt[:], st[:], mybir.ActivationFunctionType.Exp
                        )
                        if filler is not None:
                            filler(kt)
                        # [V | ones] -> out rows 0:64, den rows 64:128
                        for half, ov in ((0, ovA), (1, ovB)):
                            nc.tensor.matmul(
                                ov[:],
                                v1_sb[:, kt, pair, half],
                                pt[:, half * QW:(half + 1) * QW],
                                start=(kt == 0),
                                stop=(kt == NT - 1),
                            )
                    # normalize: rec = 1/den (broadcast across partitions is
                    # already materialized), outT = out * rec.  The custom-DVE
                    # reciprocal misbehaves on PSUM input at partition offset
                    # 64, so bounce den through SBUF first (tensor_copy with
                    # cross partition offsets is fine).
                    for half, ov in ((0, ovA), (1, ovB)):
                        p0 = 64 * half
                        den = recp.tile([64, QW], F32, tag="den")
                        nc.vector.tensor_copy(den[:], ov[64:128, :])
                        rec = recp.tile([64, QW], F32, tag="rec")
                        nc.vector.reciprocal_approx_fast(rec[:], den[:])
                        nc.vector.tensor_mul(
                            outT_sb[p0:p0 + 64, pair, q0:q0 + QW],
                            ov[0:64, :],
                            rec[:],
                        )

                # ---- schedule -------------------------------------------
                # Window (0,0) starts right after the q/k projections of x
                # block 0; its filler emits the later k-projection chunks
                # (paced to the x block DMAs), the V tiles, and at the end
                # the q chunks for windows 1-3.
                qk_proj_chunk(wqT_sb, bq_sb, qT_sb, 0, 0)
                qk_proj_chunk(wkT_sb, bk_sb, kT_sb, 0, 0)

                def fill_w00(kt):
                    # v tile exactly when its PV needs it (emitted right
                    # after this filler), so the first exp isn't pushed out
                    # by cold-p-state V projections.
                    v_proj_tile(kt)
                    if kt % 4 == 2 and kt // 4 + 1 < NW:
                        qk_proj_chunk(wkT_sb, bk_sb, kT_sb, 0, kt // 4 + 1)
                    if kt >= 13:
                        qk_proj_chunk(wqT_sb, bq_sb, qT_sb, 0, kt - 12)

                attention_window(0, 0, filler=fill_w00)

                # pair-0 windows 1..3; spread pair-1 Q/K projection across
                # their slack with a unit at kt=0 in EVERY window (covers
                # the ov-bank normalize latency at window boundaries).  The
                # k chunks go first (window (0,1) needs them all); q-ct1
                # nch3 is reserved as window (0,1)'s own kt=0 unit -- it is
                # only read by window (3,1), much later.
                proj1 = [(wkT_sb, bk_sb, kT_sb, nch) for nch in range(NW)]
                proj1 += [(wqT_sb, bq_sb, qT_sb, nch) for nch in range(NW - 1)]

                def make_fill_proj(chunks, period):
                    it = iter(chunks)

                    def fill(kt):
                        if kt % period == 0:
                            args = next(it, None)
                            if args is not None:
                                w, b, dst, nch = args
                                qk_proj_chunk(w, b, dst, 1, nch)
                    return fill

                fp = make_fill_proj(proj1, 6)
                attention_window(1, 0, filler=fp)
                attention_window(2, 0, filler=fp)
                attention_window(3, 0, filler=fp)

                # pair-1 windows with trailing y projection interleaved.
                # y window qq is ready once pair-1 window qq is normalized.
                def make_fill_y(qq_ready):
                    chunks = list(range(qq_ready * (QW // 128),
                                        (qq_ready + 1) * (QW // 128)))
                    it = iter(chunks)

                    def fill(kt):
                        if kt % 4 == 0:
                            args = next(it, None)
                            if args is not None:
                                y_proj_tile(args)
                    return fill

                fq3 = make_fill_proj(
                    [(wqT_sb, bq_sb, qT_sb, NW - 1)], 16)
                attention_window(0, 1, filler=fq3)
                attention_window(1, 1, filler=make_fill_y(0))
                attention_window(2, 1, filler=make_fill_y(1))
                attention_window(3, 1, filler=make_fill_y(2))
                for nt in range(3 * (QW // 128), NW * (QW // 128)):
                    y_proj_tile(nt)

    nc.compile()
    return nc


def kernel(x, Wq, bq, Wk, bk, Wv, bv, Wo, bo):
    x = np.asarray(x, dtype=np.float32)
    Wq = np.asarray(Wq, dtype=np.float32)
    Wk = np.asarray(Wk, dtype=np.float32)
    Wv = np.asarray(Wv, dtype=np.float32)
    Wo = np.asarray(Wo, dtype=np.float32)
    bq = np.asarray(bq, dtype=np.float32)
    bk = np.asarray(bk, dtype=np.float32)
    bv = np.asarray(bv, dtype=np.float32)
    bo = np.asarray(bo, dtype=np.float32)

    if "nc" not in _CACHE:
        _CACHE["nc"] = build_nc()
    nc = _CACHE["nc"]

    s = 2.0 / np.sqrt(8.0)  # fold bipolar *2 and score scale (1/8 split per side)
    in_maps = []
    for core in range(N_CORES):
        b = core // (N_CORES // B)
        g = core % (N_CORES // B)
        ch = slice(g * C_LOC, (g + 1) * C_LOC)
        in_maps.append({
            "xT": np.ascontiguousarray(x[b].T),
            "wqT": np.ascontiguousarray((s * Wq[ch, :]).T),
            "wkT": np.ascontiguousarray((s * Wk[ch, :]).T),
            "wvT": np.ascontiguousarray(Wv[ch, :].T),
            "woT": np.ascontiguousarray(Wo[:, ch].T).astype(ml_dtypes.bfloat16),
            "bq": ((2.0 * bq[ch] - 1.0) / np.sqrt(8.0)).astype(np.float32),
            "bk": ((2.0 * bk[ch] - 1.0) / np.sqrt(8.0)).astype(np.float32),
        })

    _CACHE["in_maps"] = in_maps
    res = run_bass_kernel_spmd(nc, in_maps, core_ids=list(range(N_CORES)))

    g_per_b = N_CORES // B
    const = (Wo @ bv + bo).astype(np.float32)  # bv folded through out-proj
    out = np.empty((B, N, D_MODEL), dtype=np.float32)
    for b in range(B):
        acc = res.results[b * g_per_b]["y"].astype(np.float32).copy()
        for g in range(1, g_per_b):
            acc += res.results[b * g_per_b + g]["y"]
        out[b] = acc + const
    return out


# revision 35
# speedup vs baseline: 1.1883x; 1.0167x over previous
"""Bipolar self-attention on 8 Trainium2 NeuronCores.

Sharding: data-parallel over batch (B=2 -> 2 groups of 4 cores), tensor-
parallel over heads within a group (16 heads -> 4 heads/core). Each core:
  - projects its head-slice of Q/K transposed ([c, n] layout) and V natural,
    with the bipolar transform (q-0.5)*2 and the 1/sqrt(Dh) score scale
    folded into the projection weights/biases host-side,
  - computes S^T = Kb Qb^T per head tile-by-tile, exponentiates (softmax
    without max subtraction -- scores are O(10), exp is safe in fp32),
  - multiplies P^T by a per-(pair, half) [V | ones] stationary block (128
    cols) so PSUM rows 0-63 hold the attention output and rows 64-127 hold
    the softmax denominator already replicated across 64 partitions.
    Matmul cost depends only on the moving dim, so this denominator
    broadcast is free,
  - normalizes with tensor_copy (den -> SBUF; the custom-DVE reciprocal
    misreads PSUM at partition offset 64) + reciprocal_approx_fast +
    tensor_mul (no DRAM broadcast roundtrip, no PSUM evacuation copy),
  - applies its slice of the output projection (row-parallel).
Host sums the 4 partial outputs per batch and adds the bias terms.

Score path (x, Wq/Wk, Q^T/K^T) stays float32r (1 cycle/row at moving>=256;
bf16 there costs ~1e-2 of the 2e-2 error budget for no cycle gain); the
post-softmax path (P, V, out, Wo) is bf16, which cuts SBUF/DMA energy and
measurably reduces DVFS throttling.  The PE executes in order, so
projection / output-projection matmuls are interleaved INTO the attention
k-tile loops (window (0,0) weaves the k-projection chunks paced to the x
block DMAs); emission is pair-major, with y(qq-1) tiles filling the pair-1
windows.  Denser schedules (v7/v9 experiments) stretched ACT exp execution
~20% via shared power/SBUF-port pressure and were net slower.
"""

import ml_dtypes
import numpy as np

import concourse.bass as bass
import concourse.tile as tile
from concourse import bacc, mybir
from concourse.bass_utils import run_bass_kernel_spmd

D_MODEL = 1024
NHEAD = 16
HEAD_DIM = 64
B = 2
N = 2048
N_CORES = 8
HEADS_PER_CORE = NHEAD // (N_CORES // B)  # 4
C_LOC = HEADS_PER_CORE * HEAD_DIM  # 256

F32 = mybir.dt.float32
F32R = mybir.dt.float32r
BF16 = mybir.dt.bfloat16

_CACHE = {}


def build_nc():
    nc = bacc.Bacc("TRN2", target_bir_lowering=False, debug=False)

    xT = nc.dram_tensor("xT", [D_MODEL, N], F32R, kind="ExternalInput")
    wqT = nc.dram_tensor("wqT", [D_MODEL, C_LOC], F32R, kind="ExternalInput")
    wkT = nc.dram_tensor("wkT", [D_MODEL, C_LOC], F32R, kind="ExternalInput")
    wvT = nc.dram_tensor("wvT", [D_MODEL, C_LOC], F32R, kind="ExternalInput")
    woT = nc.dram_tensor("woT", [C_LOC, D_MODEL], BF16, kind="ExternalInput")
    bq = nc.dram_tensor("bq", [C_LOC], F32, kind="ExternalInput")
    bk = nc.dram_tensor("bk", [C_LOC], F32, kind="ExternalInput")
    y = nc.dram_tensor("y", [N, D_MODEL], F32, kind="ExternalOutput")

    NT = N // 128          # 16 k tiles
    DC = D_MODEL // 128    # 8 contraction chunks
    CT = C_LOC // 128      # 2 local-channel tiles (= head pairs)
    QW = 512               # q window width
    NW = N // QW           # 4 q windows

    with tile.TileContext(nc) as tc:
        with (
            tc.tile_pool(name="singles", bufs=1) as singles,
            tc.tile_pool(name="pt", bufs=4) as ptp,
            tc.tile_pool(name="rec", bufs=4) as recp,
            tc.tile_pool(name="yout", bufs=3) as youtp,
        ):
            # small biases first, then the weights/x slices the first
            # projection chain needs, so the PE can start ~6us in.
            bq_sb = singles.tile([128, CT], F32)
            nc.sync.dma_start(bq_sb[:], bq.ap().rearrange("(c p) -> p c", p=128))
            bk_sb = singles.tile([128, CT], F32)
            nc.sync.dma_start(bk_sb[:], bk.ap().rearrange("(c p) -> p c", p=128))
            wqT_sb = singles.tile([128, DC, C_LOC], F32R)
            nc.sync.dma_start(wqT_sb[:], wqT.ap().rearrange("(c p) m -> p c m", p=128))
            # per-dc tiles so each projection matmul depends only on its
            # own x slice's DMA, not on the whole block landing
            xT_dcs = [singles.tile([128, N], F32R, name=f"xT{dc}")
                      for dc in range(DC)]
            xT_r = xT.ap().rearrange("(c p) n -> p c n", p=128)
            for dc in range(DC):
                nc.sync.dma_start(xT_dcs[dc][:, 0:QW], xT_r[:, dc, 0:QW])
            wkT_sb = singles.tile([128, DC, C_LOC], F32R)
            nc.sync.dma_start(wkT_sb[:], wkT.ap().rearrange("(c p) m -> p c m", p=128))
            wvT_sb = singles.tile([128, DC, C_LOC], F32R)
            nc.sync.dma_start(wvT_sb[:], wvT.ap().rearrange("(c p) m -> p c m", p=128))
            for blk in range(1, NW):
                for dc in range(DC):
                    nc.sync.dma_start(
                        xT_dcs[dc][:, blk * QW:(blk + 1) * QW],
                        xT_r[:, dc, blk * QW:(blk + 1) * QW],
                    )
            woT_sb = singles.tile([128, CT, D_MODEL], BF16)
            nc.sync.dma_start(woT_sb[:], woT.ap().rearrange("(c p) m -> p c m", p=128))

            qT_sb = singles.tile([128, CT, N], F32R)
            kT_sb = singles.tile([128, CT, N], F32R)
            # V stationary blocks: per (k-tile, pair, half) a [128, 128]
            # block [V_head (64) | ones (64)]: PV output rows 0:64 are the
            # attention output, rows 64:128 the softmax denominator
            # replicated across partitions (broadcast for free).
            v1_sb = singles.tile([128, NT, CT, 2, 128], BF16)
            ones_sb = singles.tile([128, 128], F32)
            nc.vector.memset(ones_sb[:], 1.0)
            for nt in range(NT):
                for pair in range(CT):
                    nc.vector.tensor_copy(
                        v1_sb[:, nt, pair, :, 64:128],
                        ones_sb[:].rearrange("p (h d) -> p h d", h=2),
                    )
            outT_sb = singles.tile([128, CT, N], BF16)

            # ---- emission helpers.  All PE work is emitted via closures so
            # the interleaving below is explicit.
            with (
                tc.tile_pool(name="ps512", bufs=2, space="PSUM") as psp,
                tc.tile_pool(name="st_ps", bufs=2, space="PSUM") as stp,
                tc.tile_pool(name="ov_ps", bufs=2, space="PSUM") as ovp,
            ):
                def qk_proj_chunk(w_sb, b_sb, dst, ct, nch):
                    # one 512-wide chunk: 8 matmuls + bias add
                    ps = psp.tile([128, 512], F32, tag="ps")
                    for dc in range(DC):
                        nc.tensor.matmul(
                            ps[:],
                            w_sb[:, dc, ct * 128:(ct + 1) * 128],
                            xT_dcs[dc][:, nch * 512:(nch + 1) * 512],
                            start=(dc == 0),
                            stop=(dc == DC - 1),
                        )
                    nc.vector.tensor_tensor(
                        dst[:, ct, nch * 512:(nch + 1) * 512],
                        ps[:],
                        b_sb[:, ct:ct + 1].to_broadcast((128, 512)),
                        mybir.AluOpType.add,
                    )

                def v_proj_tile(nt):
                    # V natural: v[n, c] = sum_d xT[d, n] wvT[d, c], one n tile
                    ps = psp.tile([128, 512], F32, tag="ps")
                    for dc in range(DC):
                        nc.tensor.matmul(
                            ps[:, :C_LOC],
                            xT_dcs[dc][:, nt * 128:(nt + 1) * 128],
                            wvT_sb[:, dc, :],
                            start=(dc == 0),
                            stop=(dc == DC - 1),
                        )
                    # scatter the 4 heads into their [pair, half] slots
                    nc.vector.tensor_copy(
                        v1_sb[:, nt].rearrange("p c h w -> p (c h) w")[:, :, 0:64],
                        ps[:, :C_LOC].rearrange("p (g d) -> p g d", g=4),
                    )

                def y_proj_tile(nt):
                    # output projection for one 128-row tile: ct-outer so each
                    # stationary load feeds both 512-col chunks.
                    pss = [psp.tile([128, 512], F32, tag="ps", name=f"yps{i}")
                           for i in range(D_MODEL // 512)]
                    for ct in range(CT):
                        for cok, ps in enumerate(pss):
                            nc.tensor.matmul(
                                ps[:],
                                outT_sb[:, ct, nt * 128:(nt + 1) * 128],
                                woT_sb[:, ct, cok * 512:(cok + 1) * 512],
                                start=(ct == 0),
                                stop=(ct == CT - 1),
                            )
                    for cok, ps in enumerate(pss):
                        ys = youtp.tile([128, 512], F32, tag="ys")
                        nc.vector.tensor_copy(ys[:], ps[:])
                        nc.sync.dma_start(
                            y.ap()[nt * 128:(nt + 1) * 128,
                                   cok * 512:(cok + 1) * 512],
                            ys[:],
                        )

                def attention_window(qq, pair, filler=None):
                    """One q-window of one head pair.  `filler()` is called
                    once per k-tile to emit interleaved PE work."""
                    q0 = qq * QW
                    ovA = ovp.tile([128, QW], F32, tag="ov")
                    ovB = ovp.tile([128, QW], F32, tag="ov")
                    for kt in range(NT):
                        st = stp.tile([128, 2 * QW], F32)
                        for half, p0 in ((0, 0), (1, 64)):
                            nc.tensor.matmul(
                                st[:, half * QW:(half + 1) * QW],
                                kT_sb[p0:p0 + 64, pair,
                                      kt * 128:(kt + 1) * 128],
                                qT_sb[p0:p0 + 64, pair, q0:q0 + QW],
                                start=True,
                                stop=True,
                            )
                        pt = ptp.tile([128, 2 * QW], BF16)
                        nc.scalar.activation(
                            pt[:], st[:], mybir.ActivationFunctionType.Exp
                        )
                        if filler is not None:
                            filler(kt)
                        # [V | ones] -> out rows 0:64, den rows 64:128
                        for half, ov in ((0, ovA), (1, ovB)):
                            nc.tensor.matmul(
                                ov[:],
                                v1_sb[:, kt, pair, half],
                                pt[:, half * QW:(half + 1) * QW],
                                start=(kt == 0),
                                stop=(kt == NT - 1),
                            )
                    # normalize: rec = 1/den (broadcast across partitions is
                    # already materialized), outT = out * rec.  The custom-DVE
                    # reciprocal misbehaves on PSUM input at partition offset
                    # 64, so bounce den through SBUF first (tensor_copy with
                    # cross partition offsets is fine).
                    for half, ov in ((0, ovA), (1, ovB)):
                        p0 = 64 * half
                        den = recp.tile([64, QW], F32, tag="den")
                        nc.vector.tensor_copy(den[:], ov[64:128, :])
                        rec = recp.tile([64, QW], F32, tag="rec")
                        nc.vector.reciprocal_approx_fast(rec[:], den[:])
                        nc.vector.tensor_mul(
                            outT_sb[p0:p0 + 64, pair, q0:q0 + QW],
                            ov[0:64, :],
                            rec[:],
                        )

                # ---- schedule -------------------------------------------
                # Window (0,0) starts right after the q/k projections of x
                # block 0; its filler emits the later k-projection chunks
                # (paced to the x block DMAs), the V tiles, and at the end
                # the q chunks for windows 1-3.
                qk_proj_chunk(wqT_sb, bq_sb, qT_sb, 0, 0)
                qk_proj_chunk(wkT_sb, bk_sb, kT_sb, 0, 0)

                def fill_w00(kt):
                    # v tile exactly when its PV needs it (emitted right
                    # after this filler), so the first exp isn't pushed out
                    # by cold-p-state V projections.
                    v_proj_tile(kt)
                    if kt % 4 == 2 and kt // 4 + 1 < NW:
                        qk_proj_chunk(wkT_sb, bk_sb, kT_sb, 0, kt // 4 + 1)
                    if kt >= 13:
                        qk_proj_chunk(wqT_sb, bq_sb, qT_sb, 0, kt - 12)

                attention_window(0, 0, filler=fill_w00)

                # pair-0 windows 1..3; spread pair-1 Q/K projection across
                # their slack with a unit at kt=0 in EVERY window (covers
                # the ov-bank normalize latency at window boundaries).  The
                # k chunks go first (window (0,1) needs them all); q-ct1
                # nch3 is reserved as window (0,1)'s own kt=0 unit -- it is
                # only read by window (3,1), much later.
                proj1 = [(wkT_sb, bk_sb, kT_sb, nch) for nch in range(NW)]
                proj1 += [(wqT_sb, bq_sb, qT_sb, nch) for nch in range(NW - 1)]

                def make_fill_proj(chunks, period):
                    it = iter(chunks)

                    def fill(kt):
                        if kt % period == 0:
                            args = next(it, None)
                            if args is not None:
                                w, b, dst, nch = args
                                qk_proj_chunk(w, b, dst, 1, nch)
                    return fill

                fp = make_fill_proj(proj1, 6)
                attention_window(1, 0, filler=fp)
                attention_window(2, 0, filler=fp)
                attention_window(3, 0, filler=fp)

                # pair-1 windows with trailing y projection interleaved.
                # y window qq is ready once pair-1 window qq is normalized.
                def make_fill_y(qq_ready):
                    chunks = list(range(qq_ready * (QW // 128),
                                        (qq_ready + 1) * (QW // 128)))
                    it = iter(chunks)

                    def fill(kt):
                        if kt % 4 == 0:
                            args = next(it, None)
                            if args is not None:
                                y_proj_tile(args)
                    return fill

                fq3 = make_fill_proj(
                    [(wqT_sb, bq_sb, qT_sb, NW - 1)], 16)
                attention_window(0, 1, filler=fq3)
                attention_window(1, 1, filler=make_fill_y(0))
                attention_window(2, 1, filler=make_fill_y(1))
                attention_window(3, 1, filler=make_fill_y(2))
                for nt in range(3 * (QW // 128), NW * (QW // 128)):
                    y_proj_tile(nt)

    nc.compile()
    return nc


def kernel(x, Wq, bq, Wk, bk, Wv, bv, Wo, bo):
    x = np.asarray(x, dtype=np.float32)
    Wq = np.asarray(Wq, dtype=np.float32)
    Wk = np.asarray(Wk, dtype=np.float32)
    Wv = np.asarray(Wv, dtype=np.float32)
    Wo = np.asarray(Wo, dtype=np.float32)
    bq = np.asarray(bq, dtype=np.float32)
    bk = np.asarray(bk, dtype=np.float32)
    bv = np.asarray(bv, dtype=np.float32)
    bo = np.asarray(bo, dtype=np.float32)

    if "nc" not in _CACHE:
        _CACHE["nc"] = build_nc()
    nc = _CACHE["nc"]

    s = 2.0 / np.sqrt(8.0)  # fold bipolar *2 and score scale (1/8 split per side)
    in_maps = []
    for core in range(N_CORES):
        b = core // (N_CORES // B)
        g = core % (N_CORES // B)
        ch = slice(g * C_LOC, (g + 1) * C_LOC)
        in_maps.append({
            "xT": np.ascontiguousarray(x[b].T),
            "wqT": np.ascontiguousarray((s * Wq[ch, :]).T),
            "wkT": np.ascontiguousarray((s * Wk[ch, :]).T),
            "wvT": np.ascontiguousarray(Wv[ch, :].T),
            "woT": np.ascontiguousarray(Wo[:, ch].T).astype(ml_dtypes.bfloat16),
            "bq": ((2.0 * bq[ch] - 1.0) / np.sqrt(8.0)).astype(np.float32),
            "bk": ((2.0 * bk[ch] - 1.0) / np.sqrt(8.0)).astype(np.float32),
        })

    _CACHE["in_maps"] = in_maps
    res = run_bass_kernel_spmd(nc, in_maps, core_ids=list(range(N_CORES)))

    g_per_b = N_CORES // B
    const = (Wo @ bv + bo).astype(np.float32)  # bv folded through out-proj
    out = np.empty((B, N, D_MODEL), dtype=np.float32)
    for b in range(B):
        acc = res.results[b * g_per_b]["y"].astype(np.float32).copy()
        for g in range(1, g_per_b):
            acc += res.results[b * g_per_b + g]["y"]
        out[b] = acc + const
    return out
